# revision 2
# baseline (speedup 1.0000x reference)
"""GATv2 (2-layer) Trainium2 kernel, 8-core SPMD. Self-contained. v2.

Strategy (v2 — reduced DMA/instruction overhead vs v1):
- Destination-node partition across 8 cores (12500 dst nodes each): segment
  softmax + aggregation are fully core-local (no all-reduce).
- Per core, dst nodes sorted by in-degree, tiled 128/tile (98 tiles); each
  tile has a degree cap C. Edge slot (p, j) = j-th in-edge of node p.
- Score trick: lrelu(e)*att = sign(att)*lrelu(e*|att|): node tables store
  x @ (W*|att|); per-edge score = sum_c sign_c*Lrelu(v_c) per head,
  v = xl'[src]+xr'[dst].
- v2: xl/xr interleaved in one table xlr[2g]=xl row, xlr[2g+1]=xr row:
  single 512B-desc store per node-phase chunk; idx tables pre-doubled.
- v2: pad slots gather a poison row (-1e9*sign) so exp(score)=0 exactly:
  no iota/deg masking needed.
- v2: all per-tile edge idx columns preloaded in one DMA; x_T / h loads
  chunked (2KB descs); h stored un-transposed, L2 node phase uses
  HWDGE DMA-transpose loads.
- Softmax without segment-max subtraction (scores are O(10); f32 exp).
- Layer boundary: AllGather of hidden (bf16).
"""
import numpy as np
import ml_dtypes

import jax
import concourse.bass as bass
import concourse.mybir as mybir
import concourse.tile as tile
from concourse.bass import AP
from concourse.vector_clock import ScopedClock

NC = 8
N = 100000
NPC = N // NC
NT = 98
NPCP = NT * 128          # 12544
PADROW = NC * NPCP       # 100352 -> poison row
V = 785 * 128            # 100480 table rows
F2 = 64
NEG_SLOPE = 0.2
BF = mybir.dt.bfloat16
FP = mybir.dt.float32
I32 = mybir.dt.int32


# ------------------------------------------------------------------ patches
def _drain_and_barrier_split(self, tick_clock, wait_clock):
    drain_inst = self.nc.sync.drain()
    wait_clock.add_sem_waits(
        drain_inst.ins, ScopedClock({None: tick_clock.global_clock})
    )
    si = drain_inst.ins.sync_info
    if si is not None and len(si.on_wait) > 1:
        waits = list(si.on_wait)
        ups = list(si.on_update)
        drain_inst.ins.sync_info = mybir.SyncInfo(on_wait=waits[:1], on_update=ups)
        for i in range(1, len(waits)):
            extra = self.nc.sync.drain()
            extra.ins.sync_info = mybir.SyncInfo(on_wait=waits[i:i + 1], on_update=[])
    self.nc.all_engine_barrier()
    assert self.sems is not None
    popped = self.nc._tile_sem_poison_stack.pop()
    assert popped is self._sem_poison
    self.nc.clear_and_free_semaphores(list(self.sems.allocated().values()))
    self.nc.all_engine_barrier()


tile.TileContext._drain_and_barrier = _drain_and_barrier_split


def split_waits(nc, maxw=1):
    """Walrus rejects instructions with more than ~2 sem waits: hoist excess
    waits onto EventSemaphore carriers inserted just before, on the same
    engine."""
    for fn in nc.m.functions:
        for bb in fn.blocks:
            new = []
            for inst in bb.instructions:
                si = getattr(inst, "sync_info", None)
                waits = list(si.on_wait) if si is not None and si.on_wait else []
                if len(waits) > maxw:
                    regw = [w for w in waits if w.wait_reg is not None]
                    imm = [w for w in waits if w.wait_reg is None]
                    keep_n = max(0, maxw - len(regw))
                    keep = regw + (imm[len(imm) - keep_n:] if keep_n else [])
                    extra = imm[: len(imm) - keep_n] if keep_n else imm
                    for j in range(0, len(extra), maxw):
                        new.append(mybir.InstEventSemaphore(
                            name=f"{inst.name}-wsp{j}",
                            engine=inst.engine, ins=[], outs=[],
                            sync_info=mybir.SyncInfo(
                                on_wait=extra[j:j + maxw], on_update=[]),
                        ))
                    inst.sync_info = mybir.SyncInfo(
                        on_wait=keep, on_update=list(si.on_update or []))
                new.append(inst)
            bb.instructions = new


def ap_b(ap, dims):
    """Rebuild an AP with explicit free-dim [step, count] pairs."""
    return AP(ap.tensor, ap.offset, [ap.ap[0]] + [list(d) for d in dims])


# ------------------------------------------------------------------ runner
def build_runner(nc, n_cores=NC):
    from jax.sharding import Mesh, PartitionSpec
    from jax.experimental.shard_map import shard_map
    from concourse.bass2jax import (
        _bass_exec_p, install_neuronx_cc_hook, partition_id_tensor)

    install_neuronx_cc_hook()
    partition_name = nc.partition_id_tensor.name if nc.partition_id_tensor else None
    in_names, out_names, out_avals = [], [], []
    for alloc in nc.m.functions[0].allocations:
        if not isinstance(alloc, mybir.MemoryLocationSet):
            continue
        name = alloc.memorylocations[0].name
        if alloc.kind == "ExternalInput":
            if name != partition_name:
                in_names.append(name)
        elif alloc.kind == "ExternalOutput":
            out_names.append(name)
            out_avals.append(jax.core.ShapedArray(
                tuple(alloc.tensor_shape), mybir.dt.np(alloc.dtype)))
    n_params = len(in_names)
    all_in = list(in_names) + list(out_names)
    if partition_name is not None:
        all_in.append(partition_name)

    def _body(*args):
        operands = list(args)
        if partition_name is not None:
            operands.append(partition_id_tensor())
        return tuple(_bass_exec_p.bind(
            *operands, out_avals=tuple(out_avals), in_names=tuple(all_in),
            out_names=tuple(out_names), lowering_input_output_aliases=(),
            sim_require_finite=True, sim_require_nnan=True, nc=nc))

    devices = jax.devices()[:n_cores]
    mesh = Mesh(np.asarray(devices), ("core",))
    in_specs = (PartitionSpec("core"),) * (n_params + len(out_names))
    out_specs = (PartitionSpec("core"),) * len(out_names)
    sharded = jax.jit(
        shard_map(_body, mesh=mesh, in_specs=in_specs, out_specs=out_specs,
                  check_rep=False),
        keep_unused=True)

    class Runner:
        def stage(self, in_maps):
            concat_in = [
                np.concatenate([np.asarray(in_maps[c][k]) for c in range(n_cores)], 0)
                for k in in_names]
            concat_zeros = [
                np.zeros((n_cores * a.shape[0], *a.shape[1:]), a.dtype)
                for a in out_avals]
            sh = jax.sharding.NamedSharding(mesh, PartitionSpec("core"))
            return [jax.device_put(a, sh) for a in concat_in + concat_zeros]

        def run(self, args):
            outs = sharded(*args)
            jax.block_until_ready(outs)
            return outs

        def outputs_np(self, outs):
            return [
                {name: np.asarray(outs[i]).reshape(n_cores, *out_avals[i].shape)[c]
                 for i, name in enumerate(out_names)}
                for c in range(n_cores)]

    return Runner()


# ------------------------------------------------------------------ host prep
def _prep(x, edge_index, Wl1, bl1, Wr1, br1, att1, bias1,
          Wl2, bl2, Wr2, br2, att2, bias2):
    src = np.concatenate([np.asarray(edge_index[0]), np.arange(N)]).astype(np.int64)
    dst = np.concatenate([np.asarray(edge_index[1]), np.arange(N)]).astype(np.int64)
    deg = np.bincount(dst, minlength=N)

    g = np.empty(N, np.int64)
    bucket_nodes = np.full((NC, NPCP), -1, np.int64)
    for c in range(NC):
        nodes = np.arange(c * NPC, (c + 1) * NPC)
        order = np.argsort(-deg[nodes], kind="stable")
        bn = nodes[order]
        bucket_nodes[c, :NPC] = bn
        g[bn] = c * NPCP + np.arange(NPC)

    degp = np.zeros((NC, NPCP), np.int64)
    degp[:, :NPC] = deg[bucket_nodes[:, :NPC]]
    tile_max = degp.reshape(NC, NT, 128).max(axis=(0, 2))
    caps = np.maximum(4, ((tile_max + 3) // 4) * 4).astype(np.int64)
    assert caps.max() <= 96, f"degree cap too large: {caps.max()}"
    csum = np.zeros(NT + 1, np.int64)
    csum[1:] = np.cumsum(caps)
    SC = int(csum[-1])            # total slot columns

    eorder = np.argsort(dst, kind="stable")
    ssrc = src[eorder]
    rowptr = np.zeros(N + 1, np.int64)
    rowptr[1:] = np.cumsum(deg)
    gsrc_sorted = g[ssrc]  # table row of each edge's src, grouped by dst

    # idx2[c, p, csum[t]+j] = 2*g(src of j-th in-edge of node (c,t,p)),
    # pad slots -> 2*PADROW (poison row)
    idx2 = np.full((NC, 128, SC), 2 * PADROW, np.int64)
    for c in range(NC):
        for t in range(NT):
            C = int(caps[t])
            blk = np.full((128, C), PADROW, np.int64)
            for p in range(128):
                node = bucket_nodes[c, t * 128 + p]
                if node >= 0:
                    d = int(deg[node])
                    blk[p, :d] = gsrc_sorted[rowptr[node]:rowptr[node] + d]
            idx2[c, :, csum[t]:csum[t + 1]] = 2 * blk
    idx2 = idx2.astype(np.int32)

    # xr row table index: 2*(c*NPCP + t*128 + p) + 1
    xr_idx = np.empty((NC, 128, NT), np.int32)
    for c in range(NC):
        xr_idx[c] = 2 * ((c * NPCP + np.arange(NT) * 128)[None, :] +
                         np.arange(128)[:, None]) + 1

    x_perm = np.zeros((V, 128), np.float32)
    x_perm[g] = np.asarray(x, np.float32)
    x_T = np.ascontiguousarray(x_perm.T).astype(ml_dtypes.bfloat16)

    a1 = np.asarray(att1, np.float32).reshape(-1)
    a2 = np.asarray(att2, np.float32).reshape(-1)
    aa1 = np.maximum(np.abs(a1), 1e-12)
    aa2 = np.maximum(np.abs(a2), 1e-12)
    W1_all = np.concatenate(
        [np.asarray(Wl1, np.float32) * aa1[None, :],
         np.asarray(Wr1, np.float32) * aa1[None, :]], 1).astype(ml_dtypes.bfloat16)
    W2_all = np.concatenate(
        [np.asarray(Wl2, np.float32) * aa2[None, :],
         np.asarray(Wr2, np.float32) * aa2[None, :]], 1).astype(ml_dtypes.bfloat16)
    sgn1 = np.sign(a1)
    sgn2 = np.sign(a2)
    sign1_rep = np.tile(sgn1[None, :], (128, 1)).astype(ml_dtypes.bfloat16)
    sign2_rep = np.tile(sgn2[None, :], (128, 1)).astype(ml_dtypes.bfloat16)
    ra1_rep = np.tile((1.0 / aa1)[None, :], (128, 1)).astype(np.float32)
    ra2_rep = np.tile((1.0 / aa2)[None, :], (128, 1)).astype(np.float32)
    # poison rows: xl_pad = -1e9*sign so sign*lrelu(xl_pad+xr) <= -2e8 always
    pad1 = (-1e9 * np.where(sgn1 == 0, 1.0, sgn1))[None, :].astype(ml_dtypes.bfloat16)
    pad2 = (-1e9 * np.where(sgn2 == 0, 1.0, sgn2))[None, :].astype(ml_dtypes.bfloat16)

    return dict(caps=caps, csum=csum, SC=SC, idx2=idx2,
                xr_idx=xr_idx, x_T=x_T, W1_all=W1_all, W2_all=W2_all,
                sign1_rep=sign1_rep, sign2_rep=sign2_rep, ra1_rep=ra1_rep,
                ra2_rep=ra2_rep, pad1=pad1, pad2=pad2,
                iperm=bucket_nodes)


# ------------------------------------------------------------------ program
def build_program(caps, SC, repeats=1):
    caps = [int(c) for c in caps]
    csum = np.zeros(NT + 1, np.int64)
    csum[1:] = np.cumsum(caps)
    CMAX = max(caps)
    nc = bass.Bass(num_devices=NC)
    x_T = nc.declare_dram_parameter("x_T", [128, V], BF, isOutput=False)
    idx2 = nc.declare_dram_parameter("idx2", [128, SC], I32, isOutput=False)
    xr_idx = nc.declare_dram_parameter("xr_idx", [128, NT], I32, isOutput=False)
    W1p = nc.declare_dram_parameter("W1_all", [128, 256], BF, isOutput=False)
    W2p = nc.declare_dram_parameter("W2_all", [128, 128], BF, isOutput=False)
    s1p = nc.declare_dram_parameter("sign1_rep", [128, 128], BF, isOutput=False)
    s2p = nc.declare_dram_parameter("sign2_rep", [128, 64], BF, isOutput=False)
    ra1p = nc.declare_dram_parameter("ra1_rep", [128, 128], FP, isOutput=False)
    ra2p = nc.declare_dram_parameter("ra2_rep", [128, 64], FP, isOutput=False)
    pad1p = nc.declare_dram_parameter("pad1", [1, 128], BF, isOutput=False)
    pad2p = nc.declare_dram_parameter("pad2", [1, 64], BF, isOutput=False)
    outp = nc.declare_dram_parameter("out", [NPCP, F2], FP, isOutput=True)

    xlr_tab = nc.dram_tensor("xlr_tab", [2 * V, 128], BF)
    xlr2_tab = nc.dram_tensor("xlr2_tab", [2 * V, 64], BF)
    AGC = 7                     # all-gather chunks (14 tiles each)
    agrows = NPCP // AGC        # 1792
    h_shard = nc.dram_tensor("h_shard", [NPCP, 128], BF)
    h_all = nc.dram_tensor("h_all", [AGC, NC, agrows, 128], BF,
                           addr_space="Shared")

    def psum_copy(j, out, in_):
        if j % 2 == 0:
            nc.scalar.copy(out=out, in_=in_)
        else:
            nc.vector.tensor_copy(out=out, in_=in_)

    with tile.TileContext(nc) as tc:
        with (tc.tile_pool(name="const", bufs=1) as cpool,
              tc.tile_pool(name="xr", bufs=1) as xrpool,
              tc.tile_pool(name="big", bufs=2) as bpool,
              tc.tile_pool(name="gsl", bufs=2) as gpool,
              tc.tile_pool(name="work", bufs=3) as pool,
              tc.tile_pool(name="stage", bufs=3) as spool,
              tc.tile_pool(name="psum", bufs=2, space="PSUM") as ppool):
            W1_sb = cpool.tile([128, 256], BF)
            nc.sync.dma_start(out=W1_sb[:], in_=W1p[:])
            W2_sb = cpool.tile([128, 128], BF)
            nc.sync.dma_start(out=W2_sb[:], in_=W2p[:])
            s1_sb = cpool.tile([128, 128], BF)
            nc.sync.dma_start(out=s1_sb[:], in_=s1p[:])
            s2_sb = cpool.tile([128, 64], BF)
            nc.sync.dma_start(out=s2_sb[:], in_=s2p[:])
            ra1_sb = cpool.tile([128, 128], FP)
            nc.sync.dma_start(out=ra1_sb[:], in_=ra1p[:])
            ra2_sb = cpool.tile([128, 64], FP)
            nc.sync.dma_start(out=ra2_sb[:], in_=ra2p[:])
            pad1_sb = cpool.tile([1, 128], BF)
            nc.sync.dma_start(out=pad1_sb[:], in_=pad1p[:])
            pad2_sb = cpool.tile([1, 64], BF)
            nc.sync.dma_start(out=pad2_sb[:], in_=pad2p[:])
            xri_sb = cpool.tile([128, NT], I32)
            nc.sync.dma_start(out=xri_sb[:], in_=xr_idx[:])
            xr1_sb = xrpool.tile([128, NT * 128], BF)
            xr2_sb = xrpool.tile([128, NT * 64], BF)
            it_all = xrpool.tile([128, SC], I32)

            for _rep in range(repeats):
                nc.sync.dma_start(out=it_all[:], in_=idx2[:])

                # ---- layer-1 node phase: xlr_tab[2g]=xl, [2g+1]=xr
                for nt0 in range(0, 784, 8):
                    xt = spool.tile([128, 8 * 128], BF, tag="xt1")
                    nc.sync.dma_start(
                        out=xt[:], in_=x_T[:, nt0 * 128:(nt0 + 8) * 128])
                    stg = spool.tile([128, 8 * 256], BF, tag="stg1")
                    for j in range(8):
                        ps = ppool.tile([128, 256], FP, tag="psA")
                        nc.tensor.matmul(
                            ps[:], lhsT=xt[:, j * 128:(j + 1) * 128],
                            rhs=W1_sb[:], start=True, stop=True)
                        psum_copy(j, stg[:, j * 256:(j + 1) * 256], ps[:])
                    nc.sync.dma_start(
                        out=xlr_tab[2 * nt0 * 128:2 * (nt0 + 8) * 128, :].rearrange(
                            "(j p v) f -> p j (v f)", p=128, v=2),
                        in_=stg[:].rearrange("p (j f) -> p j f", f=256))
                # tile 784 covers only pad rows: just write the poison row
                nc.sync.dma_start(
                    out=xlr_tab[2 * PADROW:2 * PADROW + 1, :], in_=pad1_sb[0:1, :])

                # ---- layer-1 edge phase (xr preloads interleaved; each slot
                # gather gets its own small tile so Pool never WAW-chains)
                for t in range(NT):
                    C = caps[t]
                    c0 = int(csum[t])
                    nc.gpsimd.indirect_dma_start(
                        out=xr1_sb[:, t * 128:(t + 1) * 128], out_offset=None,
                        in_=xlr_tab[:],
                        in_offset=bass.IndirectOffsetOnAxis(
                            ap=xri_sb[:, t:t + 1], axis=0))
                    gjs = []
                    for j in range(C):
                        gj = gpool.tile([128, 128], BF, tag=f"ga{j}")
                        nc.gpsimd.indirect_dma_start(
                            out=gj[:], out_offset=None,
                            in_=xlr_tab[:],
                            in_offset=bass.IndirectOffsetOnAxis(
                                ap=it_all[:, c0 + j:c0 + j + 1], axis=0))
                        gjs.append(gj)
                    vt = bpool.tile([128, CMAX * 128], BF, tag="v1")
                    # v = xl_g + xr  (per slot; xr shared across slots)
                    for j in range(C):
                        nc.vector.tensor_add(
                            out=vt[:, j * 128:(j + 1) * 128], in0=gjs[j][:],
                            in1=xr1_sb[:, t * 128:(t + 1) * 128])
                    nc.vector.scalar_tensor_tensor(
                        out=vt[:, :C * 128], in0=vt[:, :C * 128],
                        scalar=NEG_SLOPE, in1=vt[:, :C * 128],
                        op0=mybir.AluOpType.mult, op1=mybir.AluOpType.max)
                    vt3 = vt[:, :C * 128].rearrange("p (j f) -> p j f", f=128)
                    nc.vector.tensor_mul(
                        out=vt3, in0=vt3,
                        in1=ap_b(s1_sb[:], [[0, C], [1, 128]]))
                    # score per head: reduce the 16 channels of each head
                    sc = pool.tile([128, CMAX * 8], FP, tag="sc")
                    nc.vector.tensor_reduce(
                        sc[:, :C * 8],
                        vt[:, :C * 128].rearrange("p (j h c) -> p j h c", h=8, c=16),
                        axis=mybir.AxisListType.X, op=mybir.AluOpType.add)
                    amb = pool.tile([128, CMAX * 8], BF, tag="amb")
                    nc.scalar.activation(amb[:, :C * 8], sc[:, :C * 8],
                                         mybir.ActivationFunctionType.Exp)
                    # z = xl_g * a  (a broadcast over the 16 channels)
                    zt = bpool.tile([128, CMAX * 128], BF, tag="z1")
                    for j in range(C):
                        nc.vector.tensor_mul(
                            out=zt[:, j * 128:(j + 1) * 128].rearrange(
                                "p (h c) -> p h c", h=8, c=16),
                            in0=gjs[j][:].rearrange("p (h c) -> p h c", h=8, c=16),
                            in1=ap_b(amb[:, j * 8:(j + 1) * 8], [[1, 8], [0, 16]]))
                    agg = pool.tile([128, 128], FP, tag="agg")
                    nc.vector.tensor_reduce(
                        agg[:],
                        zt[:, :C * 128].rearrange("p (j f) -> p f j", f=128),
                        axis=mybir.AxisListType.X, op=mybir.AluOpType.add)
                    den = pool.tile([128, 8], FP, tag="den")
                    nc.vector.tensor_reduce(
                        den[:],
                        ap_b(amb[:, :C * 8], [[1, 8], [8, C]]),
                        axis=mybir.AxisListType.X, op=mybir.AluOpType.add)
                    nc.vector.tensor_scalar_max(den[:], den[:], 1e-30)
                    rec = pool.tile([128, 8], FP, tag="rec")
                    nc.vector.reciprocal(rec[:], den[:])
                    hb = pool.tile([128, 128], FP, tag="hb")
                    nc.vector.tensor_mul(
                        out=hb[:], in0=agg[:],
                        in1=ap_b(rec[:], [[1, 8], [0, 16]]))
                    nc.vector.tensor_mul(out=hb[:], in0=hb[:], in1=ra1_sb[:])
                    # ELU
                    xm = pool.tile([128, 128], FP, tag="xm")
                    nc.vector.tensor_scalar_min(xm[:], hb[:], 0.0)
                    nc.scalar.activation(xm[:], xm[:],
                                         mybir.ActivationFunctionType.Exp)
                    xp = pool.tile([128, 128], FP, tag="xp")
                    nc.vector.tensor_scalar_max(xp[:], hb[:], 0.0)
                    hf = pool.tile([128, 128], BF, tag="hf")
                    nc.vector.scalar_tensor_tensor(
                        out=hf[:], in0=xm[:], scalar=-1.0, in1=xp[:],
                        op0=mybir.AluOpType.add, op1=mybir.AluOpType.add)
                    nc.sync.dma_start(
                        out=h_shard[t * 128:(t + 1) * 128, :], in_=hf[:])

                # ---- all-gather hidden (chunked so early chunks overlap the
                # tail of the layer-1 edge phase)
                for a in range(AGC):
                    nc.gpsimd.collective_compute(
                        "AllGather", mybir.AluOpType.bypass,
                        replica_groups=[list(range(NC))],
                        ins=[h_shard[a * agrows:(a + 1) * agrows, :]],
                        outs=[h_all[a]])

                # ---- layer-2 node phase (transpose loads from h_all; 7-tile
                # chunks so each load sits inside one all-gather chunk)
                for q in range(NC):
                    for tq0 in range(0, NT, 7):
                        k = 7
                        ntt0 = q * NT + tq0
                        a, loc = divmod(tq0, 14)
                        ht = spool.tile([128, 7 * 128], BF, tag="xt2")
                        nc.sync.dma_start(
                            out=ht[:, :k * 128],
                            in_=h_all[a, q, loc * 128:(loc + k) * 128, :],
                            transpose=True)
                        stg = spool.tile([128, 7 * 128], BF, tag="stg2")
                        for j in range(k):
                            ps = ppool.tile([128, 128], FP, tag="psB")
                            nc.tensor.matmul(
                                ps[:], lhsT=ht[:, j * 128:(j + 1) * 128],
                                rhs=W2_sb[:], start=True, stop=True)
                            psum_copy(j, stg[:, j * 128:(j + 1) * 128], ps[:])
                        nc.sync.dma_start(
                            out=xlr2_tab[2 * ntt0 * 128:2 * (ntt0 + k) * 128, :]
                            .rearrange("(j p v) f -> p j (v f)", p=128, v=2),
                            in_=stg[:, :k * 128].rearrange("p (j f) -> p j f", f=128))
                nc.sync.dma_start(
                    out=xlr2_tab[2 * PADROW:2 * PADROW + 1, :], in_=pad2_sb[0:1, :])

                # ---- layer-2 edge phase (same structure as layer 1)
                for t in range(NT):
                    C = caps[t]
                    c0 = int(csum[t])
                    nc.gpsimd.indirect_dma_start(
                        out=xr2_sb[:, t * 64:(t + 1) * 64], out_offset=None,
                        in_=xlr2_tab[:],
                        in_offset=bass.IndirectOffsetOnAxis(
                            ap=xri_sb[:, t:t + 1], axis=0))
                    gjs = []
                    for j in range(C):
                        gj = gpool.tile([128, 64], BF, tag=f"gb{j}")
                        nc.gpsimd.indirect_dma_start(
                            out=gj[:], out_offset=None,
                            in_=xlr2_tab[:],
                            in_offset=bass.IndirectOffsetOnAxis(
                                ap=it_all[:, c0 + j:c0 + j + 1], axis=0))
                        gjs.append(gj)
                    vt = bpool.tile([128, CMAX * 64], BF, tag="v2")
                    for j in range(C):
                        nc.vector.tensor_add(
                            out=vt[:, j * 64:(j + 1) * 64], in0=gjs[j][:],
                            in1=xr2_sb[:, t * 64:(t + 1) * 64])
                    nc.vector.scalar_tensor_tensor(
                        out=vt[:, :C * 64], in0=vt[:, :C * 64],
                        scalar=NEG_SLOPE, in1=vt[:, :C * 64],
                        op0=mybir.AluOpType.mult, op1=mybir.AluOpType.max)
                    vt3 = vt[:, :C * 64].rearrange("p (j f) -> p j f", f=64)
                    nc.vector.tensor_mul(
                        out=vt3, in0=vt3,
                        in1=ap_b(s2_sb[:], [[0, C], [1, 64]]))
                    sc = pool.tile([128, CMAX], FP, tag="sc2")
                    nc.vector.tensor_reduce(
                        sc[:, :C], vt3, axis=mybir.AxisListType.X,
                        op=mybir.AluOpType.add)
                    amb = pool.tile([128, CMAX], BF, tag="amb2")
                    nc.scalar.activation(amb[:, :C], sc[:, :C],
                                         mybir.ActivationFunctionType.Exp)
                    zt = bpool.tile([128, CMAX * 64], BF, tag="z2")
                    for j in range(C):
                        nc.vector.tensor_mul(
                            out=zt[:, j * 64:(j + 1) * 64], in0=gjs[j][:],
                            in1=ap_b(amb[:, j:j + 1], [[0, 64]]))
                    agg = pool.tile([128, 64], FP, tag="agg2")
                    nc.vector.tensor_reduce(
                        agg[:],
                        zt[:, :C * 64].rearrange("p (j f) -> p f j", f=64),
                        axis=mybir.AxisListType.X, op=mybir.AluOpType.add)
                    den = pool.tile([128, 1], FP, tag="den2")
                    nc.vector.tensor_reduce(
                        den[:], amb[:, :C], axis=mybir.AxisListType.X,
                        op=mybir.AluOpType.add)
                    nc.vector.tensor_scalar_max(den[:], den[:], 1e-30)
                    rec = pool.tile([128, 1], FP, tag="rec2")
                    nc.vector.reciprocal(rec[:], den[:])
                    ot = pool.tile([128, 64], FP, tag="ot")
                    nc.vector.scalar_tensor_tensor(
                        out=ot[:], in0=agg[:], scalar=rec[:], in1=ra2_sb[:],
                        op0=mybir.AluOpType.mult, op1=mybir.AluOpType.mult)
                    nc.sync.dma_start(
                        out=outp[t * 128:(t + 1) * 128, :], in_=ot[:])

    split_waits(nc)
    return nc


_CACHE = {}


def get_runner(prep, repeats=1):
    key = (tuple(int(c) for c in prep["caps"]), repeats)
    if key not in _CACHE:
        nc = build_program(prep["caps"], prep["SC"], repeats=repeats)
        _CACHE[key] = build_runner(nc, NC)
    return _CACHE[key]


def make_in_maps(prep):
    return [{
        "x_T": np.asarray(prep["x_T"]),
        "idx2": prep["idx2"][c],
        "xr_idx": prep["xr_idx"][c],
        "W1_all": prep["W1_all"],
        "W2_all": prep["W2_all"],
        "sign1_rep": prep["sign1_rep"],
        "sign2_rep": prep["sign2_rep"],
        "ra1_rep": prep["ra1_rep"],
        "ra2_rep": prep["ra2_rep"],
        "pad1": prep["pad1"],
        "pad2": prep["pad2"],
    } for c in range(NC)]


def unshard(prep, res):
    out = np.zeros((N, F2), np.float32)
    for c in range(NC):
        rows = res[c]["out"]
        nodes = prep["iperm"][c]
        valid = nodes >= 0
        out[nodes[valid]] = rows[np.nonzero(valid)[0]]
    return out


def kernel(**inputs) -> np.ndarray:
    prep = _prep(**inputs)
    r = get_runner(prep, repeats=1)
    try:
        res = r.outputs_np(r.run(r.stage(make_in_maps(prep))))
    except Exception:
        # transient device/tunnel hiccups recover on retry
        res = r.outputs_np(r.run(r.stage(make_in_maps(prep))))
    return unshard(prep, res)
